# revision 33
# baseline (speedup 1.0000x reference)
"""ClassAttention (decode-style single-query attention) on 8 TRN2 NeuronCores.

Math (per batch b):
    kv = x @ Wkv              # [N, 2*H*D], k half cols 0:1024, v half 1024:2048
    q  = x[0] @ Wq            # [H*D]  (CLS token only)
    logits[t, h] = scale * sum_d q[h,d] * k[t, h*64+d]
    attn = softmax_t(logits)
    cls[h,d] = sum_t attn[t,h] * v[t, h*64+d]
    out = cls @ Wproj + bproj

v4 (hybrid of the proven v2 pipeline skeleton and the large-N math):
  - k / v are never materialized: logits fold into wkf and the attention
    output is reassociated through x.
  - Sweep matmuls are N=512 with tiny 16-col stationaries:
      logits^T[h, t] : lhsT = wkf block [128, 16], moving = x^T [128, 4, 128]
      r[h, c]        : lhsT = e block   [128, 16], moving = x   [128, 512]
    r and sum_t(exp) accumulate in PSUM chains (the sums chain rides the
    same stationary as the r matmuls: one extra N=1 matmul against ones).
  - x^T per 4-tile group: even groups via PE transposes (copies alternate
    DVE/ACT), odd groups via one XBAR transposing DMA on the sync queue.
  - Queue plan: SWDGE carries ONLY x (16 x 2MB contiguous-per-partition
    cast-loads). Wq/Wk stage f32 on the scalar queue and Wv/Wproj on the
    sync queue, all cast to resident bf16 on ACT early (the ACT queue is
    drained before the first exp; data-gated DMAs never sit on it).
  - Softmax runs without max-subtraction (logits are O(1)); the 1/sum(exp)
    normalization is applied to the tiny r[h, c] tensor at finalize.

Sharding: pure data-parallel over B: 16 batches / 8 cores = 2 per core.
Weights are replicated; each core returns its [2, 1024] output shard.
"""

import numpy as np

import concourse.bass as bass
import concourse.mybir as mybir
import concourse.tile as tile
from concourse import bacc
from concourse.bass_utils import run_bass_kernel_spmd
from concourse.masks import make_identity

F32 = mybir.dt.float32
BF16 = mybir.dt.bfloat16

B, SEQ, C = 16, 4096, 1024
H, D = 16, 64
SCALE = D ** -0.5  # 0.125
N_CORES = 8
BPC = B // N_CORES          # batches per core
CB = C // 128               # 8 contraction blocks
GRP = 4                     # t-tiles per group
NG = SEQ // (GRP * 128)     # 8 groups per batch


def _build():
    nc = bacc.Bacc(
        "TRN2", target_bir_lowering=False, debug=False, num_devices=N_CORES
    )
    x_ap = nc.dram_tensor("x", [BPC, SEQ, C], F32, kind="ExternalInput").ap()
    wq_ap = nc.dram_tensor("Wq", [C, H * D], F32, kind="ExternalInput").ap()
    wkv_ap = nc.dram_tensor("Wkv", [C, 2 * H * D], F32, kind="ExternalInput").ap()
    wp_ap = nc.dram_tensor("Wproj", [H * D, C], F32, kind="ExternalInput").ap()
    bp_ap = nc.dram_tensor("bproj", [C], F32, kind="ExternalInput").ap()
    out_ap = nc.dram_tensor("out", [BPC, C], F32, kind="ExternalOutput").ap()

    with tile.TileContext(nc) as tc:
        _emit(nc, tc, x_ap, wq_ap, wkv_ap, wp_ap, bp_ap, out_ap)
    nc.compile()
    return nc


def _emit(nc, tc, x_ap, wq_ap, wkv_ap, wp_ap, bp_ap, out_ap):
    with tc.tile_pool(name="consts", bufs=1) as consts:
        wv_bf = consts.tile([128, CB * 1024], BF16)
        wp_bf = consts.tile([128, CB * 1024], BF16)
        bproj_sb = consts.tile([1, C], F32)
        nc.sync.dma_start(bproj_sb[:], bp_ap[:].unsqueeze(0))

        id16_bf = consts.tile([16, 16], BF16)
        make_identity(nc, id16_bf[:])
        id16_f = consts.tile([16, 16], F32)
        make_identity(nc, id16_f[:])
        id128 = consts.tile([128, 128], BF16)
        make_identity(nc, id128[:])
        ones_bf = consts.tile([128, 1], BF16)
        nc.vector.memset(ones_bf[:], 1.0)

        wkf_bf = [
            consts.tile([128, CB * H], BF16, tag=f"wkf{b}", name=f"wkf{b}")
            for b in range(BPC)
        ]

        with (
            tc.tile_pool(name="xbf", bufs=5) as xbf_pool,
            tc.tile_pool(name="xt", bufs=3) as xt_pool,
            tc.tile_pool(name="wstage", bufs=3) as wstage,
            tc.tile_pool(name="esb", bufs=4) as esb_pool,
            tc.tile_pool(name="ebf", bufs=4) as ebf_pool,
            tc.tile_pool(name="small", bufs=1) as small,
        ):
            # ---------------- prefix ----------------
            with (
                tc.tile_pool(name="wqk", bufs=1) as wqk,
                tc.tile_pool(name="pre", bufs=1) as pre,
                tc.tile_pool(name="fold", bufs=2) as fold_pool,
                tc.tile_pool(name="qps", bufs=2, space="PSUM") as qps,
                tc.tile_pool(name="qbps", bufs=1, space="PSUM") as qbps,
                tc.tile_pool(name="xtps", bufs=1, space="PSUM") as xtps,
            ):
                # CLS rows (single-descriptor loads on sync), PE-transposed
                xcls_nat = pre.tile([16, C], F32)
                nc.vector.memset(xcls_nat[:], 0.0)
                for b in range(BPC):
                    nc.sync.dma_start(xcls_nat[b : b + 1, :], x_ap[b, 0:1, :])
                xclsT_ps = xtps.tile([128, 128], F32, tag="xT")
                for g in range(CB):
                    nc.tensor.transpose(
                        xclsT_ps[:, g * H : (g + 1) * H],
                        xcls_nat[:, g * 128 : (g + 1) * 128],
                        id16_f[:],
                    )
                xclsT = pre.tile([128, 128], BF16)
                nc.vector.tensor_copy(xclsT[:], xclsT_ps[:])

                wq_bf = wqk.tile([128, CB * 1024], BF16, tag="wq")
                wk_bf = wqk.tile([128, CB * 1024], BF16, tag="wk")
                for dst, src_ap, c0 in ((wq_bf, wq_ap, 0), (wk_bf, wkv_ap, 0)):
                    for g in range(CB):
                        nc.gpsimd.dma_start(
                            dst[:, g * 1024 : (g + 1) * 1024],
                            src_ap[g * 128 : (g + 1) * 128, c0 : c0 + 1024],
                        )

                # selector rows: sel[b][k, m] = SCALE if k == b else 0
                sel0 = pre.tile([BPC, 128], F32, tag="sel0", name="sel0")
                nc.vector.memset(sel0[:], 0.0)
                nc.vector.memset(sel0[0:1, :], SCALE)
                sel1 = pre.tile([BPC, 128], F32, tag="sel1", name="sel1")
                nc.vector.memset(sel1[:], SCALE)
                nc.vector.memset(sel1[0:1, :], 0.0)
                sel = [sel0, sel1]

                # q for both batches at once: [2, 512] chains over c blocks
                q_ps = [
                    qps.tile([BPC, 512], F32, tag="q", name=f"qps{ch}")
                    for ch in range(2)
                ]
                for g in range(CB):
                    for ch in range(2):
                        nc.tensor.matmul(
                            q_ps[ch][:],
                            xclsT[:, g * H : g * H + BPC],
                            wq_bf[:, g * 1024 + ch * 512 : g * 1024 + (ch + 1) * 512],
                            start=(g == 0),
                            stop=(g == CB - 1),
                        )
                q2_sb = pre.tile([BPC, C], F32)
                for ch in range(2):
                    nc.vector.tensor_copy(
                        q2_sb[:, ch * 512 : (ch + 1) * 512], q_ps[ch][:]
                    )

                # qb[b] = scale * q[b] broadcast down partitions (bf16)
                qb_sb = [
                    pre.tile([128, C], BF16, tag=f"qb{b}", name=f"qb{b}")
                    for b in range(BPC)
                ]
                for b in range(BPC):
                    for ch in range(2):
                        qb_ps = qbps.tile([128, 512], F32, tag="qb")
                        nc.tensor.matmul(
                            qb_ps[:],
                            sel[b][:],
                            q2_sb[:, ch * 512 : (ch + 1) * 512],
                            start=True,
                            stop=True,
                        )
                        nc.vector.tensor_copy(
                            qb_sb[b][:, ch * 512 : (ch + 1) * 512], qb_ps[:]
                        )

                # wkf[b][c, (g h)] = scale * sum_d q[b,(h,d)] * Wk[c,(h,d)]
                for b in range(BPC):
                    for g in range(CB):
                        prod = fold_pool.tile([128, H * D], BF16, tag="prod")
                        nc.vector.tensor_mul(
                            prod[:], wk_bf[:, g * 1024 : (g + 1) * 1024],
                            qb_sb[b][:],
                        )
                        wkf_g = fold_pool.tile([128, H], F32, tag="wkfg")
                        nc.vector.tensor_reduce(
                            wkf_g[:].unsqueeze(2),
                            prod[:].rearrange("p (h d) -> p h d", d=D),
                            axis=mybir.AxisListType.X,
                            op=mybir.AluOpType.add,
                        )
                        nc.vector.tensor_copy(
                            wkf_bf[b][:, g * H : (g + 1) * H], wkf_g[:]
                        )

            # ---- SWDGE queue: x group cast-loads only ----
            x_tiles = {}
            for b in range(BPC):
                for tg in range(NG):
                    x_bf = xbf_pool.tile([128, GRP * C], BF16, tag="x",
                                         name="x_bf")
                    nc.gpsimd.dma_start(
                        x_bf[:].rearrange("p (i c) -> p i c", i=GRP),
                        x_ap[b, tg * GRP * 128 : (tg + 1) * GRP * 128, :]
                        .rearrange("(i t) c -> t i c", i=GRP),
                    )
                    x_tiles[(b, tg)] = x_bf


            # ---------------- sweeps ----------------
            with (
                tc.tile_pool(name="lgps", bufs=3, space="PSUM") as lgps,
                tc.tile_pool(name="raps", bufs=2, space="PSUM") as raps,
                tc.tile_pool(name="smps", bufs=1, space="PSUM") as smps,
                tc.tile_pool(name="e16ps", bufs=2, space="PSUM") as e16ps,
            ):
                state = {}

                def emit_xt(b, tg):
                    """x^T for group tg via one XBAR transposing DMA:
                    [128, (i g) 128] with m = i*CB + g."""
                    x_bf = x_tiles[(b, tg)]
                    xt = xt_pool.tile([128, GRP * CB * 128], BF16, tag="xt",
                                      name="xt")
                    nc.sync.dma_start(
                        xt[:].rearrange("p (m c) -> p m c", c=128),
                        x_bf[:], transpose=True,
                    )
                    return xt

                def emit_logits(b, tg, xt):
                    wkf = wkf_bf[b]
                    lg = lgps.tile([16, 512], F32, tag="lg")
                    xt_v = xt[:].rearrange("p (i g c) -> p i g c", i=GRP, g=CB)
                    for g in range(CB):
                        nc.tensor.matmul(
                            lg[:],
                            wkf[:, g * H : (g + 1) * H],
                            xt_v[:, :, g, :],
                            start=(g == 0),
                            stop=(g == CB - 1),
                        )
                    e_sb = esb_pool.tile([16, 512], BF16, tag="e")
                    nc.scalar.activation(
                        e_sb[:], lg[:], mybir.ActivationFunctionType.Exp
                    )
                    return e_sb

                def emit_racc(b, tg, e_sb):
                    st = state[b]
                    x_bf = x_tiles[(b, tg)]
                    eT = e16ps.tile([128, CB * H], BF16, tag="e16")
                    for j in range(GRP):
                        nc.tensor.transpose(
                            eT[:, j * H : (j + 1) * H],
                            e_sb[:, j * 128 : (j + 1) * 128],
                            id16_bf[:],
                        )
                    e_bf = ebf_pool.tile([128, GRP * H], BF16, tag="ebf")
                    nc.vector.tensor_copy(e_bf[:], eT[:, 0 : GRP * H])
                    for j in range(GRP):
                        first = tg == 0 and j == 0
                        last = tg == NG - 1 and j == GRP - 1
                        lhsT = e_bf[:, j * H : (j + 1) * H]
                        nc.tensor.matmul(
                            st["rA"][:], lhsT, x_bf[:, j * C : j * C + 512],
                            start=first, stop=last,
                        )
                        nc.tensor.matmul(
                            st["rB"][:], lhsT, x_bf[:, j * C + 512 : (j + 1) * C],
                            start=first, stop=last,
                        )
                        nc.tensor.matmul(
                            st["sm"][0:16, 0:1], lhsT, ones_bf[:],
                            start=first, stop=last,
                        )

                # Wv/Wproj blocks: f32 stage on sync, bf16 cast on the DVE;
                # emitted a few blocks per sweep group so neither the ACT
                # queue nor the prefix HBM window carries them
                wv_blocks = [(wv_bf, wkv_ap, 1024, g) for g in range(CB)]
                wp_blocks = [(wp_bf, wp_ap, 0, g) for g in range(CB)]
                wpend = wv_blocks + wp_blocks

                def emit_wblocks(n):
                    for _ in range(n):
                        if not wpend:
                            return
                        dst, src_ap, c0, g = wpend.pop(0)
                        wst = wstage.tile([128, 1024], F32, tag="wst")
                        nc.sync.dma_start(
                            wst[:],
                            src_ap[g * 128 : (g + 1) * 128, c0 : c0 + 1024],
                        )
                        nc.vector.tensor_copy(
                            dst[:, g * 1024 : (g + 1) * 1024], wst[:]
                        )

                def sweep(b):
                    st = state.setdefault(b, {})
                    st["rA"] = raps.tile([16, 512], F32, tag="ra", name="rA")
                    st["rB"] = raps.tile([16, 512], F32, tag="ra", name="rB")
                    st["sm"] = smps.tile([16, 512], F32, tag="sm", name="sm")
                    pend = None
                    xt = emit_xt(b, 0)
                    for tg in range(NG):
                        e_sb = emit_logits(b, tg, xt)
                        if tg + 1 < NG:
                            xt = emit_xt(b, tg + 1)
                        if pend is not None:
                            emit_racc(b, *pend)
                        pend = (tg, e_sb)
                        if b == 0 and tg >= 2:
                            emit_wblocks(3)
                    emit_racc(b, *pend)

                def finalize(b):
                    st = state[b]
                    sums = small.tile([16, 1], F32, tag="sums", name="sums")
                    nc.vector.tensor_copy(sums[:], st["sm"][0:16, 0:1])
                    rec = small.tile([16, 1], F32, tag="rec", name="rec")
                    nc.vector.reciprocal(rec[:], sums[:])
                    r_bf = small.tile([16, C], BF16, tag="rbf", name="rbf")
                    nc.vector.tensor_scalar_mul(
                        r_bf[:, 0:512], st["rA"][:], rec[:]
                    )
                    nc.vector.tensor_scalar_mul(
                        r_bf[:, 512:1024], st["rB"][:], rec[:]
                    )

                    rT_ps = e16ps.tile([128, CB * H], BF16, tag="e16")
                    for g in range(CB):
                        nc.tensor.transpose(
                            rT_ps[:, g * H : (g + 1) * H],
                            r_bf[:, g * 128 : (g + 1) * 128],
                            id16_bf[:],
                        )
                    rT_bf = small.tile([128, CB * H], BF16, tag="rTb",
                                       name="rTb")
                    nc.vector.tensor_copy(rT_bf[:], rT_ps[:])

                    cls_bf = small.tile([16, C], BF16, tag="cls", name="cls")
                    for ch in range(2):
                        cls_ps = lgps.tile([16, 512], F32, tag="lg")
                        for g in range(CB):
                            nc.tensor.matmul(
                                cls_ps[:],
                                rT_bf[:, g * H : (g + 1) * H],
                                wv_bf[:, g * 1024 + ch * 512 :
                                      g * 1024 + (ch + 1) * 512],
                                start=(g == 0),
                                stop=(g == CB - 1),
                            )
                        nc.vector.tensor_copy(
                            cls_bf[:, ch * 512 : (ch + 1) * 512], cls_ps[:]
                        )

                    # diagonal pick: clsv[hd] = cls_bf[hd//64, hd]
                    aT = e16ps.tile([128, CB * H], BF16, tag="e16")
                    for g in range(CB):
                        nc.tensor.transpose(
                            aT[:, g * H : (g + 1) * H],
                            cls_bf[:, g * 128 : (g + 1) * 128],
                            id16_bf[:],
                        )
                    clsv_bf = small.tile([128, CB], BF16, tag="cv", name="cv")
                    for g in range(CB):
                        for half in range(2):
                            rows = slice(64 * half, 64 * half + 64)
                            col = g * H + 2 * g + half
                            nc.vector.tensor_copy(
                                clsv_bf[rows, g : g + 1], aT[rows, col : col + 1]
                            )

                    o_sb = small.tile([1, C], F32, tag="osb", name="osb")
                    for ch in range(2):
                        o_ps = lgps.tile([16, 512], F32, tag="lg")
                        for g in range(CB):
                            nc.tensor.matmul(
                                o_ps[0:1, :],
                                clsv_bf[:, g : g + 1],
                                wp_bf[:, g * 1024 + ch * 512 :
                                      g * 1024 + (ch + 1) * 512],
                                start=(g == 0),
                                stop=(g == CB - 1),
                            )
                        nc.vector.tensor_add(
                            o_sb[0:1, ch * 512 : (ch + 1) * 512],
                            o_ps[0:1, :],
                            bproj_sb[0:1, ch * 512 : (ch + 1) * 512],
                        )
                    nc.sync.dma_start(out_ap[b : b + 1, :], o_sb[:])

                sweep(0)
                finalize(0)
                sweep(1)
                finalize(1)


_CACHED = None


def _get_program():
    global _CACHED
    if _CACHED is None:
        _CACHED = _build()
    return _CACHED


def kernel(x, Wq, Wkv, Wproj, bproj, _trace=False):
    x = np.ascontiguousarray(np.asarray(x, dtype=np.float32))
    Wq = np.ascontiguousarray(np.asarray(Wq, dtype=np.float32))
    Wkv = np.ascontiguousarray(np.asarray(Wkv, dtype=np.float32))
    Wproj = np.ascontiguousarray(np.asarray(Wproj, dtype=np.float32))
    bproj = np.ascontiguousarray(np.asarray(bproj, dtype=np.float32))

    nc = _get_program()
    in_maps = [
        {
            "x": x[cid * BPC : (cid + 1) * BPC],
            "Wq": Wq,
            "Wkv": Wkv,
            "Wproj": Wproj,
            "bproj": bproj,
        }
        for cid in range(N_CORES)
    ]
    res = run_bass_kernel_spmd(
        nc, in_maps, core_ids=list(range(N_CORES)), trace=_trace
    )
    out = np.concatenate([res.results[cid]["out"] for cid in range(N_CORES)], axis=0)
    if _trace:
        kernel.last_exec_time_ns = res.exec_time_ns
        kernel.last_results = res
    return out.reshape(B, 1, C)


# revision 34
# speedup vs baseline: 1.2050x; 1.2050x over previous
"""ClassAttention (decode-style single-query attention) on 8 TRN2 NeuronCores.

Math (per batch b):
    kv = x @ Wkv              # [N, 2*H*D], k half cols 0:1024, v half 1024:2048
    q  = x[0] @ Wq            # [H*D]  (CLS token only)
    logits[t, h] = scale * sum_d q[h,d] * k[t, h*64+d]
    attn = softmax_t(logits)
    cls[h,d] = sum_t attn[t,h] * v[t, h*64+d]
    out = cls @ Wproj + bproj

v4 (hybrid of the proven v2 pipeline skeleton and the large-N math):
  - k / v are never materialized: logits fold into wkf and the attention
    output is reassociated through x.
  - Sweep matmuls are N=512 with tiny 16-col stationaries:
      logits^T[h, t] : lhsT = wkf block [128, 16], moving = x^T [128, 4, 128]
      r[h, c]        : lhsT = e block   [128, 16], moving = x   [128, 512]
    r and sum_t(exp) accumulate in PSUM chains (the sums chain rides the
    same stationary as the r matmuls: one extra N=1 matmul against ones).
  - x^T per 4-tile group: even groups via PE transposes (copies alternate
    DVE/ACT), odd groups via one XBAR transposing DMA on the sync queue.
  - Queue plan: SWDGE carries ONLY x (16 x 2MB contiguous-per-partition
    cast-loads). Wq/Wk stage f32 on the scalar queue and Wv/Wproj on the
    sync queue, all cast to resident bf16 on ACT early (the ACT queue is
    drained before the first exp; data-gated DMAs never sit on it).
  - Softmax runs without max-subtraction (logits are O(1)); the 1/sum(exp)
    normalization is applied to the tiny r[h, c] tensor at finalize.

Sharding: pure data-parallel over B: 16 batches / 8 cores = 2 per core.
Weights are replicated; each core returns its [2, 1024] output shard.
"""

import numpy as np

import concourse.bass as bass
import concourse.mybir as mybir
import concourse.tile as tile
from concourse import bacc
from concourse.bass_utils import run_bass_kernel_spmd
from concourse.masks import make_identity

F32 = mybir.dt.float32
BF16 = mybir.dt.bfloat16

B, SEQ, C = 16, 4096, 1024
H, D = 16, 64
SCALE = D ** -0.5  # 0.125
N_CORES = 8
BPC = B // N_CORES          # batches per core
CB = C // 128               # 8 contraction blocks
GRP = 4                     # t-tiles per group
NG = SEQ // (GRP * 128)     # 8 groups per batch


def _build():
    nc = bacc.Bacc(
        "TRN2", target_bir_lowering=False, debug=False, num_devices=N_CORES
    )
    x_ap = nc.dram_tensor("x", [BPC, SEQ, C], F32, kind="ExternalInput").ap()
    wq_ap = nc.dram_tensor("Wq", [C, H * D], F32, kind="ExternalInput").ap()
    wkv_ap = nc.dram_tensor("Wkv", [C, 2 * H * D], F32, kind="ExternalInput").ap()
    wp_ap = nc.dram_tensor("Wproj", [H * D, C], F32, kind="ExternalInput").ap()
    bp_ap = nc.dram_tensor("bproj", [C], F32, kind="ExternalInput").ap()
    out_ap = nc.dram_tensor("out", [BPC, C], F32, kind="ExternalOutput").ap()

    with tile.TileContext(nc) as tc:
        _emit(nc, tc, x_ap, wq_ap, wkv_ap, wp_ap, bp_ap, out_ap)
    nc.compile()
    return nc


def _emit(nc, tc, x_ap, wq_ap, wkv_ap, wp_ap, bp_ap, out_ap):
    with tc.tile_pool(name="consts", bufs=1) as consts:
        wv_bf = consts.tile([128, CB * 1024], BF16)
        wp_bf = consts.tile([128, CB * 1024], BF16)
        bproj_sb = consts.tile([1, C], F32)
        nc.sync.dma_start(bproj_sb[:], bp_ap[:].unsqueeze(0))

        id16_bf = consts.tile([16, 16], BF16)
        make_identity(nc, id16_bf[:])
        id16_f = consts.tile([16, 16], F32)
        make_identity(nc, id16_f[:])
        id128 = consts.tile([128, 128], BF16)
        make_identity(nc, id128[:])
        ones_bf = consts.tile([128, 1], BF16)
        nc.vector.memset(ones_bf[:], 1.0)

        wkf_bf = [
            consts.tile([128, CB * H], BF16, tag=f"wkf{b}", name=f"wkf{b}")
            for b in range(BPC)
        ]

        with (
            tc.tile_pool(name="xbf", bufs=7) as xbf_pool,
            tc.tile_pool(name="xt", bufs=3) as xt_pool,
            tc.tile_pool(name="wstage", bufs=3) as wstage,
            tc.tile_pool(name="esb", bufs=4) as esb_pool,
            tc.tile_pool(name="ebf", bufs=4) as ebf_pool,
            tc.tile_pool(name="small", bufs=1) as small,
        ):
            # ---------------- prefix ----------------
            with (
                tc.tile_pool(name="wqk", bufs=1) as wqk,
                tc.tile_pool(name="pre", bufs=1) as pre,
                tc.tile_pool(name="fold", bufs=2) as fold_pool,
                tc.tile_pool(name="qps", bufs=2, space="PSUM") as qps,
                tc.tile_pool(name="qbps", bufs=1, space="PSUM") as qbps,
                tc.tile_pool(name="xtps", bufs=1, space="PSUM") as xtps,
            ):
                # CLS rows (single-descriptor loads on sync), PE-transposed
                xcls_nat = pre.tile([16, C], F32)
                nc.vector.memset(xcls_nat[:], 0.0)
                for b in range(BPC):
                    nc.sync.dma_start(xcls_nat[b : b + 1, :], x_ap[b, 0:1, :])
                xclsT_ps = xtps.tile([128, 128], F32, tag="xT")
                for g in range(CB):
                    nc.tensor.transpose(
                        xclsT_ps[:, g * H : (g + 1) * H],
                        xcls_nat[:, g * 128 : (g + 1) * 128],
                        id16_f[:],
                    )
                xclsT = pre.tile([128, 128], BF16)
                nc.vector.tensor_copy(xclsT[:], xclsT_ps[:])

                wq_bf = wqk.tile([128, CB * 1024], BF16, tag="wq")
                wk_bf = wqk.tile([128, CB * 1024], BF16, tag="wk")
                for dst, src_ap, c0 in ((wq_bf, wq_ap, 0), (wk_bf, wkv_ap, 0)):
                    for g in range(CB):
                        wst = wstage.tile([128, 1024], F32, tag="wst")
                        nc.scalar.dma_start(
                            wst[:],
                            src_ap[g * 128 : (g + 1) * 128, c0 : c0 + 1024],
                        )
                        nc.scalar.copy(dst[:, g * 1024 : (g + 1) * 1024], wst[:])

                # selector rows: sel[b][k, m] = SCALE if k == b else 0
                sel0 = pre.tile([BPC, 128], F32, tag="sel0", name="sel0")
                nc.vector.memset(sel0[:], 0.0)
                nc.vector.memset(sel0[0:1, :], SCALE)
                sel1 = pre.tile([BPC, 128], F32, tag="sel1", name="sel1")
                nc.vector.memset(sel1[:], SCALE)
                nc.vector.memset(sel1[0:1, :], 0.0)
                sel = [sel0, sel1]

                # q for both batches at once: [2, 512] chains over c blocks
                q_ps = [
                    qps.tile([BPC, 512], F32, tag="q", name=f"qps{ch}")
                    for ch in range(2)
                ]
                for g in range(CB):
                    for ch in range(2):
                        nc.tensor.matmul(
                            q_ps[ch][:],
                            xclsT[:, g * H : g * H + BPC],
                            wq_bf[:, g * 1024 + ch * 512 : g * 1024 + (ch + 1) * 512],
                            start=(g == 0),
                            stop=(g == CB - 1),
                        )
                q2_sb = pre.tile([BPC, C], F32)
                for ch in range(2):
                    nc.vector.tensor_copy(
                        q2_sb[:, ch * 512 : (ch + 1) * 512], q_ps[ch][:]
                    )

                # qb[b] = scale * q[b] broadcast down partitions (bf16)
                qb_sb = [
                    pre.tile([128, C], BF16, tag=f"qb{b}", name=f"qb{b}")
                    for b in range(BPC)
                ]
                for b in range(BPC):
                    for ch in range(2):
                        qb_ps = qbps.tile([128, 512], F32, tag="qb")
                        nc.tensor.matmul(
                            qb_ps[:],
                            sel[b][:],
                            q2_sb[:, ch * 512 : (ch + 1) * 512],
                            start=True,
                            stop=True,
                        )
                        nc.vector.tensor_copy(
                            qb_sb[b][:, ch * 512 : (ch + 1) * 512], qb_ps[:]
                        )

                # wkf[b][c, (g h)] = scale * sum_d q[b,(h,d)] * Wk[c,(h,d)]
                for b in range(BPC):
                    for g in range(CB):
                        prod = fold_pool.tile([128, H * D], BF16, tag="prod")
                        nc.vector.tensor_mul(
                            prod[:], wk_bf[:, g * 1024 : (g + 1) * 1024],
                            qb_sb[b][:],
                        )
                        wkf_g = fold_pool.tile([128, H], F32, tag="wkfg")
                        nc.vector.tensor_reduce(
                            wkf_g[:].unsqueeze(2),
                            prod[:].rearrange("p (h d) -> p h d", d=D),
                            axis=mybir.AxisListType.X,
                            op=mybir.AluOpType.add,
                        )
                        nc.vector.tensor_copy(
                            wkf_bf[b][:, g * H : (g + 1) * H], wkf_g[:]
                        )

            # ---- SWDGE queue: x group cast-loads only ----
            x_tiles = {}
            for b in range(BPC):
                for tg in range(NG):
                    x_bf = xbf_pool.tile([128, GRP * C], BF16, tag="x",
                                         name="x_bf")
                    nc.gpsimd.dma_start(
                        x_bf[:].rearrange("p (i c) -> p i c", i=GRP),
                        x_ap[b, tg * GRP * 128 : (tg + 1) * GRP * 128, :]
                        .rearrange("(i t) c -> t i c", i=GRP),
                    )
                    x_tiles[(b, tg)] = x_bf


            # ---------------- sweeps ----------------
            with (
                tc.tile_pool(name="trps", bufs=2, space="PSUM") as trps,
                tc.tile_pool(name="lgps", bufs=2, space="PSUM") as lgps,
                tc.tile_pool(name="raps", bufs=2, space="PSUM") as raps,
                tc.tile_pool(name="smps", bufs=1, space="PSUM") as smps,
                tc.tile_pool(name="e16ps", bufs=1, space="PSUM") as e16ps,
            ):
                state = {}

                def emit_xt(b, tg):
                    """x^T for group tg: [128, (i g) 128] with m = i*CB + g.
                    Even groups on the PE, odd groups via the XBAR DMA."""
                    x_bf = x_tiles[(b, tg)]
                    xt = xt_pool.tile([128, GRP * CB * 128], BF16, tag="xt",
                                      name="xt")
                    if tg % 2 == 1:
                        nc.sync.dma_start(
                            xt[:].rearrange("p (m c) -> p m c", c=128),
                            x_bf[:], transpose=True,
                        )
                    else:
                        for k in range(GRP * CB // 4):
                            tps = trps.tile([128, 512], BF16, tag="tps")
                            for j in range(4):
                                blk = k * 4 + j
                                nc.tensor.transpose(
                                    tps[:, j * 128 : (j + 1) * 128],
                                    x_bf[:, blk * 128 : (blk + 1) * 128],
                                    id128[:],
                                )
                            if k % 2 == 0:
                                nc.vector.tensor_copy(
                                    xt[:, k * 512 : (k + 1) * 512], tps[:]
                                )
                            else:
                                nc.scalar.copy(
                                    xt[:, k * 512 : (k + 1) * 512], tps[:]
                                )
                    return xt

                def emit_logits(b, tg, xt):
                    wkf = wkf_bf[b]
                    lg = lgps.tile([16, 512], F32, tag="lg")
                    xt_v = xt[:].rearrange("p (i g c) -> p i g c", i=GRP, g=CB)
                    for g in range(CB):
                        nc.tensor.matmul(
                            lg[:],
                            wkf[:, g * H : (g + 1) * H],
                            xt_v[:, :, g, :],
                            start=(g == 0),
                            stop=(g == CB - 1),
                        )
                    e_sb = esb_pool.tile([16, 512], BF16, tag="e")
                    nc.scalar.activation(
                        e_sb[:], lg[:], mybir.ActivationFunctionType.Exp
                    )
                    return e_sb

                def emit_racc(b, tg, e_sb):
                    st = state[b]
                    x_bf = x_tiles[(b, tg)]
                    eT = e16ps.tile([128, CB * H], BF16, tag="e16")
                    for j in range(GRP):
                        nc.tensor.transpose(
                            eT[:, j * H : (j + 1) * H],
                            e_sb[:, j * 128 : (j + 1) * 128],
                            id16_bf[:],
                        )
                    e_bf = ebf_pool.tile([128, GRP * H], BF16, tag="ebf")
                    nc.vector.tensor_copy(e_bf[:], eT[:, 0 : GRP * H])
                    for j in range(GRP):
                        first = tg == 0 and j == 0
                        last = tg == NG - 1 and j == GRP - 1
                        lhsT = e_bf[:, j * H : (j + 1) * H]
                        nc.tensor.matmul(
                            st["rA"][:], lhsT, x_bf[:, j * C : j * C + 512],
                            start=first, stop=last,
                        )
                        nc.tensor.matmul(
                            st["rB"][:], lhsT, x_bf[:, j * C + 512 : (j + 1) * C],
                            start=first, stop=last,
                        )
                        nc.tensor.matmul(
                            st["sm"][0:16, 0:1], lhsT, ones_bf[:],
                            start=first, stop=last,
                        )

                # Wv/Wproj blocks: f32 stage on sync, bf16 cast on the DVE;
                # emitted a few blocks per sweep group so neither the ACT
                # queue nor the prefix HBM window carries them
                wv_blocks = [(wv_bf, wkv_ap, 1024, g) for g in range(CB)]
                wp_blocks = [(wp_bf, wp_ap, 0, g) for g in range(CB)]
                wpend = wv_blocks + wp_blocks

                def emit_wblocks(n):
                    for _ in range(n):
                        if not wpend:
                            return
                        dst, src_ap, c0, g = wpend.pop(0)
                        wst = wstage.tile([128, 1024], F32, tag="wst")
                        nc.sync.dma_start(
                            wst[:],
                            src_ap[g * 128 : (g + 1) * 128, c0 : c0 + 1024],
                        )
                        nc.vector.tensor_copy(
                            dst[:, g * 1024 : (g + 1) * 1024], wst[:]
                        )

                def sweep(b):
                    st = state.setdefault(b, {})
                    st["rA"] = raps.tile([16, 512], F32, tag="ra", name="rA")
                    st["rB"] = raps.tile([16, 512], F32, tag="ra", name="rB")
                    st["sm"] = smps.tile([16, 512], F32, tag="sm", name="sm")
                    pend = None
                    xt = emit_xt(b, 0)
                    for tg in range(NG):
                        e_sb = emit_logits(b, tg, xt)
                        if tg + 1 < NG:
                            xt = emit_xt(b, tg + 1)
                        if pend is not None:
                            emit_racc(b, *pend)
                        pend = (tg, e_sb)
                        if b == 0 and tg >= 2:
                            emit_wblocks(3)
                    emit_racc(b, *pend)

                def finalize(b):
                    st = state[b]
                    sums = small.tile([16, 1], F32, tag="sums", name="sums")
                    nc.vector.tensor_copy(sums[:], st["sm"][0:16, 0:1])
                    rec = small.tile([16, 1], F32, tag="rec", name="rec")
                    nc.vector.reciprocal(rec[:], sums[:])
                    r_bf = small.tile([16, C], BF16, tag="rbf", name="rbf")
                    nc.vector.tensor_scalar_mul(
                        r_bf[:, 0:512], st["rA"][:], rec[:]
                    )
                    nc.vector.tensor_scalar_mul(
                        r_bf[:, 512:1024], st["rB"][:], rec[:]
                    )

                    rT_ps = e16ps.tile([128, CB * H], BF16, tag="e16")
                    for g in range(CB):
                        nc.tensor.transpose(
                            rT_ps[:, g * H : (g + 1) * H],
                            r_bf[:, g * 128 : (g + 1) * 128],
                            id16_bf[:],
                        )
                    rT_bf = small.tile([128, CB * H], BF16, tag="rTb",
                                       name="rTb")
                    nc.vector.tensor_copy(rT_bf[:], rT_ps[:])

                    cls_bf = small.tile([16, C], BF16, tag="cls", name="cls")
                    for ch in range(2):
                        cls_ps = lgps.tile([16, 512], F32, tag="lg")
                        for g in range(CB):
                            nc.tensor.matmul(
                                cls_ps[:],
                                rT_bf[:, g * H : (g + 1) * H],
                                wv_bf[:, g * 1024 + ch * 512 :
                                      g * 1024 + (ch + 1) * 512],
                                start=(g == 0),
                                stop=(g == CB - 1),
                            )
                        nc.vector.tensor_copy(
                            cls_bf[:, ch * 512 : (ch + 1) * 512], cls_ps[:]
                        )

                    # diagonal pick: clsv[hd] = cls_bf[hd//64, hd]
                    aT = e16ps.tile([128, CB * H], BF16, tag="e16")
                    for g in range(CB):
                        nc.tensor.transpose(
                            aT[:, g * H : (g + 1) * H],
                            cls_bf[:, g * 128 : (g + 1) * 128],
                            id16_bf[:],
                        )
                    clsv_bf = small.tile([128, CB], BF16, tag="cv", name="cv")
                    for g in range(CB):
                        for half in range(2):
                            rows = slice(64 * half, 64 * half + 64)
                            col = g * H + 2 * g + half
                            nc.vector.tensor_copy(
                                clsv_bf[rows, g : g + 1], aT[rows, col : col + 1]
                            )

                    o_sb = small.tile([1, C], F32, tag="osb", name="osb")
                    for ch in range(2):
                        o_ps = lgps.tile([16, 512], F32, tag="lg")
                        for g in range(CB):
                            nc.tensor.matmul(
                                o_ps[0:1, :],
                                clsv_bf[:, g : g + 1],
                                wp_bf[:, g * 1024 + ch * 512 :
                                      g * 1024 + (ch + 1) * 512],
                                start=(g == 0),
                                stop=(g == CB - 1),
                            )
                        nc.vector.tensor_add(
                            o_sb[0:1, ch * 512 : (ch + 1) * 512],
                            o_ps[0:1, :],
                            bproj_sb[0:1, ch * 512 : (ch + 1) * 512],
                        )
                    nc.sync.dma_start(out_ap[b : b + 1, :], o_sb[:])

                sweep(0)
                finalize(0)
                sweep(1)
                finalize(1)


_CACHED = None


def _get_program():
    global _CACHED
    if _CACHED is None:
        _CACHED = _build()
    return _CACHED


def kernel(x, Wq, Wkv, Wproj, bproj, _trace=False):
    x = np.ascontiguousarray(np.asarray(x, dtype=np.float32))
    Wq = np.ascontiguousarray(np.asarray(Wq, dtype=np.float32))
    Wkv = np.ascontiguousarray(np.asarray(Wkv, dtype=np.float32))
    Wproj = np.ascontiguousarray(np.asarray(Wproj, dtype=np.float32))
    bproj = np.ascontiguousarray(np.asarray(bproj, dtype=np.float32))

    nc = _get_program()
    in_maps = [
        {
            "x": x[cid * BPC : (cid + 1) * BPC],
            "Wq": Wq,
            "Wkv": Wkv,
            "Wproj": Wproj,
            "bproj": bproj,
        }
        for cid in range(N_CORES)
    ]
    res = run_bass_kernel_spmd(
        nc, in_maps, core_ids=list(range(N_CORES)), trace=_trace
    )
    out = np.concatenate([res.results[cid]["out"] for cid in range(N_CORES)], axis=0)
    if _trace:
        kernel.last_exec_time_ns = res.exec_time_ns
        kernel.last_results = res
    return out.reshape(B, 1, C)


# revision 35
# speedup vs baseline: 1.6202x; 1.3446x over previous
"""ClassAttention (decode-style single-query attention) on 8 TRN2 NeuronCores.

Math (per batch b):
    kv = x @ Wkv              # [N, 2*H*D], k half cols 0:1024, v half 1024:2048
    q  = x[0] @ Wq            # [H*D]  (CLS token only)
    logits[t, h] = scale * sum_d q[h,d] * k[t, h*64+d]
    attn = softmax_t(logits)
    cls[h,d] = sum_t attn[t,h] * v[t, h*64+d]
    out = cls @ Wproj + bproj

Key restructuring (v2):
  - k is never materialized: logits = x @ wk_fold, with
    wk_fold[c,h] = scale * sum_d q[h,d] * Wk[c, h*64+d]   (folded per batch).
  - v is never materialized either: attention output is reassociated as
    xaT[c,h] = sum_t exp[t,h] * x[t,c]  (accumulated TRANSPOSED on the PE
    with x's natural layout as the moving operand and exp[128t,16h] as the
    tiny stationary), then cls[h,:] = diag-pick of (xaT_n @ Wv).
  - Softmax runs without max-subtraction (logits are O(1) by construction)
    and the 1/sum(exp) normalization is deferred to the tiny cls tensor.
  - sum_t exp[t,h] rides the same PSUM accumulation as xaT via a ones-column
    matmul (no vector work in the sweep).

Pipelining (the point of v2):
  - ALL DMA cast-loads (f32->bf16) go on the single SWDGE (gpsimd) queue in
    bandwidth-optimal order: Wq, Wk first (they gate the q-fold), then batch
    0's x groups, then Wv/Wproj (needed only at finalize), then batch 1's x.
    Loads self-gate on pool buffers and stream continuously.
  - X-bar transposes (for the logits operand x^T) all on the sync queue,
    arrival-gated, overlapping everything.
  - The PE queue is software-pipelined with depth 2: the attention-accumulate
    matmuls of tile i are emitted after the logits matmuls of tile i+2, so
    the in-order PE queue never stalls on the Scalar EXP dependency.

Sharding: pure data-parallel over B: 16 batches / 8 cores = 2 per core.
Weights are replicated; each core returns its [2, 1024] output shard.
"""

import numpy as np

import concourse.bass as bass
import concourse.mybir as mybir
import concourse.tile as tile
from concourse import bacc
from concourse.bass_utils import run_bass_kernel_spmd
from concourse.masks import make_identity

F32 = mybir.dt.float32
BF16 = mybir.dt.bfloat16

B, SEQ, C = 16, 4096, 1024
H, D = 16, 64
SCALE = D ** -0.5  # 0.125
N_CORES = 8
BPC = B // N_CORES          # batches per core
T_TILES = SEQ // 128        # 32 sequence tiles of 128 rows per batch
CB = C // 128               # 8 contraction blocks
GRP = 4                     # t-tiles per DMA group
NG = T_TILES // GRP         # 8 groups per batch


def _build():
    nc = bacc.Bacc(
        "TRN2", target_bir_lowering=False, debug=False, num_devices=N_CORES
    )
    x_ap = nc.dram_tensor("x", [BPC, SEQ, C], F32, kind="ExternalInput").ap()
    wq_ap = nc.dram_tensor("Wq", [C, H * D], F32, kind="ExternalInput").ap()
    wkv_ap = nc.dram_tensor("Wkv", [C, 2 * H * D], F32, kind="ExternalInput").ap()
    wp_ap = nc.dram_tensor("Wproj", [H * D, C], F32, kind="ExternalInput").ap()
    bp_ap = nc.dram_tensor("bproj", [C], F32, kind="ExternalInput").ap()
    out_ap = nc.dram_tensor("out", [BPC, C], F32, kind="ExternalOutput").ap()

    with tile.TileContext(nc) as tc:
        _emit(nc, tc, x_ap, wq_ap, wkv_ap, wp_ap, bp_ap, out_ap)
    nc.compile()
    return nc


def _emit(nc, tc, x_ap, wq_ap, wkv_ap, wp_ap, bp_ap, out_ap):
    with tc.tile_pool(name="consts", bufs=1) as consts:
        wq_bf = consts.tile([128, CB * 1024], BF16)
        wk_bf = consts.tile([128, CB * 1024], BF16)
        wv_bf = consts.tile([128, CB * 1024], BF16)
        wp_bf = consts.tile([128, CB * 1024], BF16)

        bproj_sb = consts.tile([1, C], F32)
        nc.scalar.dma_start(bproj_sb[:], bp_ap[:].unsqueeze(0))

        # CLS rows of x, transposed on load: xcls[p, b*CB+g] = x[b, 0, g*128+p]
        xcls_bf = consts.tile([128, BPC * CB], BF16)
        for b in range(BPC):
            nc.gpsimd.dma_start(
                xcls_bf[:, b * CB : (b + 1) * CB],
                x_ap[b, 0:1, :].rearrange("o (g p) -> p (o g)", p=128),
            )

        # ---- SWDGE cast-load order: Wq, Wk | x b0 | x b1 ----
        def load_w(dst, src_ap, c0):
            for g in range(CB):
                nc.gpsimd.dma_start(
                    dst[:, g * 1024 : (g + 1) * 1024],
                    src_ap[g * 128 : (g + 1) * 128, c0 : c0 + 1024],
                )

        load_w(wq_bf, wq_ap, 0)
        load_w(wk_bf, wkv_ap, 0)

        ones_bf = consts.tile([128, 128], BF16)      # sums-of-exp matmul lhsT
        nc.vector.memset(ones_bf[:], 1.0)
        sc_row = consts.tile([1, 128], BF16)         # scale * ones: q broadcast
        nc.vector.memset(sc_row[:], SCALE)
        identity = consts.tile([16, 16], F32)        # PE-transpose identity (fin)
        make_identity(nc, identity[:])
        id128 = consts.tile([128, 128], BF16)        # PE-transpose identity (x)
        make_identity(nc, id128[:])

        with (
            tc.tile_pool(name="xbf", bufs=8) as xbf_pool,
            tc.tile_pool(name="xt", bufs=3) as xt_pool,
            tc.tile_pool(name="wstage", bufs=2) as wstage,
            tc.tile_pool(name="fold", bufs=2) as fold_pool,
            tc.tile_pool(name="persist", bufs=1) as persist,
            tc.tile_pool(name="small", bufs=2) as small,
            tc.tile_pool(name="exp", bufs=4) as exp_pool,
        ):
            # Wv/Wproj: only needed at finalize. Load f32 on the sync HWDGE
            # queue (concurrent with the SWDGE stream) and cast on the DVE.
            for dst, src_ap, c0 in ((wv_bf, wkv_ap, 1024), (wp_bf, wp_ap, 0)):
                for g in range(CB):
                    wst = wstage.tile([128, 1024], F32, tag="wst")
                    nc.sync.dma_start(
                        wst[:],
                        src_ap[g * 128 : (g + 1) * 128, c0 : c0 + 1024],
                    )
                    # cast on the ACT engine: scalar is idle until the sweep
                    nc.scalar.copy(dst[:, g * 1024 : (g + 1) * 1024], wst[:])

            # ---- all x group loads (gpsimd, buffer-gated) ----
            xbf_tiles = {}
            for b in range(BPC):
                for tg in range(NG):
                    x_bf = xbf_pool.tile([128, GRP * C], BF16, tag="xbf")
                    nc.gpsimd.dma_start(
                        x_bf[:].rearrange("p (i c) -> p i c", i=GRP),
                        x_ap[b, tg * GRP * 128 : (tg + 1) * GRP * 128, :].rearrange(
                            "(i t) c -> t i c", i=GRP
                        ),
                    )
                    xbf_tiles[(b, tg)] = x_bf

            # ---- q + wk_fold per batch ----
            wkfs = []
            with tc.tile_pool(name="qpsum", bufs=1, space="PSUM") as qpsum:
                for b in range(BPC):
                    wkfs.append(
                        _emit_qfold(
                            nc, b, qpsum, xcls_bf, wq_bf, wk_bf, sc_row,
                            fold_pool, persist, small,
                        )
                    )

            # ---- sweeps + finalize ----
            with (
                tc.tile_pool(name="lgpsum", bufs=2, space="PSUM") as lgpsum,
                tc.tile_pool(name="xapsum", bufs=2, space="PSUM") as xapsum,
                tc.tile_pool(name="trpsum", bufs=2, space="PSUM") as trpsum,
                tc.tile_pool(name="finpsum", bufs=1, space="PSUM") as finpsum,
            ):
                for b in range(BPC):
                    acc = _emit_sweep(
                        nc, b, wkfs[b], ones_bf, id128, xbf_tiles,
                        lgpsum, xapsum, trpsum, xt_pool, exp_pool, persist,
                    )
                    _emit_finalize(
                        nc, b, acc, wv_bf, wp_bf, bproj_sb, identity,
                        small, finpsum, out_ap,
                    )


def _emit_qfold(nc, b, qpsum, xcls_bf, wq_bf, wk_bf, sc_row, fold_pool, persist, small):
    """q = x_cls @ Wq; wk_fold[c,h] = scale * sum_d q[h,d]*Wk[c,(h,d)]."""
    q_ps = qpsum.tile([1, H * D], F32, tag="qps")
    for g in range(CB):
        lt = xcls_bf[:, b * CB + g : b * CB + g + 1]
        for ch in range(2):
            nc.tensor.matmul(
                q_ps[0:1, ch * 512 : (ch + 1) * 512],
                lt,
                wq_bf[:, g * 1024 + ch * 512 : g * 1024 + (ch + 1) * 512],
                start=(g == 0),
                stop=(g == CB - 1),
            )
    q_sb = small.tile([1, H * D], BF16, tag="qsb")
    nc.vector.tensor_copy(q_sb[:], q_ps[:])

    # qb[c_p, hd] = scale * q[hd]  (outer product broadcast down partitions)
    qb_ps = qpsum.tile([128, H * D], F32, tag="qbps")
    for ch in range(2):
        nc.tensor.matmul(
            qb_ps[:, ch * 512 : (ch + 1) * 512],
            sc_row[0:1, :],
            q_sb[0:1, ch * 512 : (ch + 1) * 512],
            start=True,
            stop=True,
        )
    qb_sb = persist.tile([128, H * D], BF16, tag=f"qb{b}")
    nc.vector.tensor_copy(qb_sb[:], qb_ps[:])

    wkf_bf = persist.tile([128, CB * H], BF16, tag=f"wkf{b}")
    for g in range(CB):
        prod = fold_pool.tile([128, H * D], BF16, tag="prod")
        nc.vector.tensor_mul(
            prod[:], wk_bf[:, g * 1024 : (g + 1) * 1024], qb_sb[:]
        )
        wkf_g = fold_pool.tile([128, H], F32, tag="wkfg")
        nc.vector.tensor_reduce(
            wkf_g[:].unsqueeze(2),
            prod[:].rearrange("p (h d) -> p h d", d=D),
            axis=mybir.AxisListType.X,
            op=mybir.AluOpType.add,
        )
        nc.vector.tensor_copy(wkf_bf[:, g * H : (g + 1) * H], wkf_g[:])
    return wkf_bf


def _emit_sweep(nc, b, wkf_bf, ones_bf, id128, xbf_tiles, lgpsum, xapsum,
                trpsum, xt_pool, exp_pool, persist):
    """logits -> exp -> transposed attention accumulate, PE-pipelined depth 2.

    Each tile's 9 matmuls are SINGLE-SHOT (start+stop) into a rotating PSUM
    tile — interleaved open accumulation chains within one PSUM bank corrupt
    each other on TRN2 — and the running sum lives in SBUF f32, accumulated
    on the (otherwise idle) vector engine.  Layout of the [128, 144] tile:
    cols g*16:(g+1)*16 hold xaT block g; cols 128:144 hold sum_t exp[t,:]
    broadcast down all partitions (ones-matmul) so one vector add covers
    the whole tile."""
    acc = persist.tile([128, CB * H + H], F32, tag=f"acc{b}")

    def emit_ptrans(x_bf):
        """x^T for the logits GEMM, on the PE (the DMA pipe is the scarce
        resource; X-bar transposes double its traffic).  PSUM->SBUF copies
        alternate between the vector and scalar engines."""
        xt = xt_pool.tile([128, GRP * CB * 128], BF16, tag="xt")
        for k in range(GRP * CB // 4):
            tps = trpsum.tile([128, 512], BF16, tag="tps")
            for j in range(4):
                blk = k * 4 + j
                nc.tensor.transpose(
                    tps[:, j * 128 : (j + 1) * 128],
                    x_bf[:, blk * 128 : (blk + 1) * 128],
                    id128[:],
                )
            if k % 2 == 0:
                nc.vector.tensor_copy(xt[:, k * 512 : (k + 1) * 512], tps[:])
            else:
                nc.scalar.copy(xt[:, k * 512 : (k + 1) * 512], tps[:])
        return xt

    def emit_xat(e, x_bf, i, ti):
        xat = xapsum.tile([128, CB * H + H], F32, tag="xat")
        for g in range(CB):
            nc.tensor.matmul(
                xat[:, g * H : (g + 1) * H],
                x_bf[:, (i * CB + g) * 128 : (i * CB + g + 1) * 128],
                e[:],
                start=True, stop=True,
            )
        nc.tensor.matmul(
            xat[:, CB * H : CB * H + H],
            ones_bf[:, 0:128],
            e[:],
            start=True, stop=True,
        )
        if ti == 0:
            nc.vector.tensor_copy(acc[:], xat[:])
        else:
            nc.vector.tensor_add(acc[:], acc[:], xat[:])

    pending = []
    xt = emit_ptrans(xbf_tiles[(b, 0)])
    for tg in range(NG):
        x_bf, cur_xt = xbf_tiles[(b, tg)], xt
        for i in range(GRP):
            ti = tg * GRP + i
            lg = lgpsum.tile([128, H], F32, tag="lg")
            for g in range(CB):
                nc.tensor.matmul(
                    lg[:],
                    cur_xt[:, (i * CB + g) * 128 : (i * CB + g + 1) * 128],
                    wkf_bf[:, g * H : (g + 1) * H],
                    start=(g == 0), stop=(g == CB - 1),
                )
            e = exp_pool.tile([128, H], BF16, tag="exp")
            nc.scalar.activation(e[:], lg[:], mybir.ActivationFunctionType.Exp)
            pending.append((e, x_bf, i, ti))
            if len(pending) > 2:
                emit_xat(*pending.pop(0))
        if tg + 1 < NG:
            xt = emit_ptrans(xbf_tiles[(b, tg + 1)])
    for args in pending:
        emit_xat(*args)
    return acc


def _emit_finalize(nc, b, acc, wv_bf, wp_bf, bproj_sb, identity, small, finpsum, out_ap):
    """cls = diag(xaT_n @ Wv), out = cls @ Wproj + bproj."""
    rec_sb = small.tile([1, H], F32, tag="rec")
    nc.vector.reciprocal(rec_sb[:], acc[0:1, CB * H : CB * H + H])
    rec_t = small.tile([16, 1], F32, tag="rec_t")
    nc.sync.dma_start(rec_t[:], rec_sb[:])  # [1,16] -> [16,1]

    xaT_bf = small.tile([128, CB * H], BF16, tag="xaT")
    nc.vector.tensor_copy(xaT_bf[:], acc[:, 0 : CB * H])

    # cls candidates: cls_ps[h, hd] = sum_c xaT[c, h] * Wv[c, hd]
    # (chunked [16,512] chains: only one PSUM bank is left for this tag)
    cls_sb = small.tile([16, 1024], F32, tag="cls_sb")
    for ch in range(2):
        cls_ps = finpsum.tile([16, 512], F32, tag="fin2")
        for g in range(CB):
            nc.tensor.matmul(
                cls_ps[:],
                xaT_bf[:, g * H : (g + 1) * H],
                wv_bf[:, g * 1024 + ch * 512 : g * 1024 + (ch + 1) * 512],
                start=(g == 0), stop=(g == CB - 1),
            )
        # normalize by 1/sum(exp) per head (h is the partition dim here)
        nc.vector.tensor_scalar_mul(
            cls_sb[:, ch * 512 : (ch + 1) * 512], cls_ps[:], rec_t[:]
        )

    # cls^T via PE transpose, then diagonal pick into [hd%128, hd//128]
    accT_ps = finpsum.tile([128, 128], F32, tag="fin1")
    for g in range(CB):
        nc.tensor.transpose(
            accT_ps[:, g * 16 : (g + 1) * 16],
            cls_sb[:, g * 128 : (g + 1) * 128],
            identity[0:16, 0:16],
        )
    cls_bf = small.tile([128, 8], BF16, tag="cls_bf")
    for h in range(16):
        g, half = h // 2, h % 2
        rows = slice(64 * half, 64 * half + 64)
        nc.vector.tensor_copy(
            cls_bf[rows, g : g + 1],
            accT_ps[rows, g * 16 + h : g * 16 + h + 1],
        )

    # out = cls @ Wproj + bproj
    o_sb = small.tile([1, C], F32, tag="osb")
    for ch in range(2):
        o_ps = finpsum.tile([1, 512], F32, tag="fin2")
        for g in range(CB):
            nc.tensor.matmul(
                o_ps[:],
                cls_bf[:, g : g + 1],
                wp_bf[:, g * 1024 + ch * 512 : g * 1024 + (ch + 1) * 512],
                start=(g == 0), stop=(g == CB - 1),
            )
        nc.vector.tensor_add(
            o_sb[0:1, ch * 512 : (ch + 1) * 512], o_ps[:],
            bproj_sb[0:1, ch * 512 : (ch + 1) * 512],
        )
    nc.sync.dma_start(out_ap[b : b + 1, :], o_sb[:])


_CACHED = None


def _get_program():
    global _CACHED
    if _CACHED is None:
        _CACHED = _build()
    return _CACHED


def kernel(x, Wq, Wkv, Wproj, bproj, _trace=False):
    x = np.ascontiguousarray(np.asarray(x, dtype=np.float32))
    Wq = np.ascontiguousarray(np.asarray(Wq, dtype=np.float32))
    Wkv = np.ascontiguousarray(np.asarray(Wkv, dtype=np.float32))
    Wproj = np.ascontiguousarray(np.asarray(Wproj, dtype=np.float32))
    bproj = np.ascontiguousarray(np.asarray(bproj, dtype=np.float32))

    nc = _get_program()
    in_maps = [
        {
            "x": x[cid * BPC : (cid + 1) * BPC],
            "Wq": Wq,
            "Wkv": Wkv,
            "Wproj": Wproj,
            "bproj": bproj,
        }
        for cid in range(N_CORES)
    ]
    res = run_bass_kernel_spmd(
        nc, in_maps, core_ids=list(range(N_CORES)), trace=_trace
    )
    out = np.concatenate([res.results[cid]["out"] for cid in range(N_CORES)], axis=0)
    if _trace:
        kernel.last_exec_time_ns = res.exec_time_ns
        kernel.last_results = res
    return out.reshape(B, 1, C)

